# revision 34
# baseline (speedup 1.0000x reference)
"""Trainium2 Bass kernel for causal degree-2 polynomial attention.

The reference module is chunked linear attention with kernel weight
(q.k)^2, which is mathematically exact causal polynomial attention:

    out_q = sum_{k<=q} (Q_q.K_k)^2 V_k / (EPS + sum_{k<=q} (Q_q.K_k)^2)

Sharding: 16 (batch, head) pairs across 8 cores -> 2 pairs/core, fully
data-parallel (matches the chunk-local-cumsum hint; no collectives).

Host-side prep (part of the shard/layout step): Q^T / K^T / [V|1] are
laid out in bf16 exactly as the device consumes them, so the device
does no transposes and no casts:
  - qT2 [128, 2048]: Q^T duplicated on both partition halves (feeds the
    two concurrent K=64 PE tiles)
  - kT2 [128, 1024]: col group a holds K^T of key block 2a (top half)
    and 2a+1 (bottom half)
  - v1  [128, 16*65]: [V_block | ones] per key block (ones col computes
    the normalizer Z on the same matmul)

Per-core device algorithm (two pairs interleaved to fill stalls):
  - D'[k, q] = K Q^T per (512-query block i, 256-key dpair a) as two
    concurrent row-tiled K=64 bf16 matmuls, narrowed to causal cols.
  - exit PSUM -> bf16 SBUF with squaring, greedily balanced between
    ACT (direct square) and DVE copy + bf16 2X multiply; psd/c2 tiles
    are [128, 2, 512] so every step (including partial diagonal ones)
    exits both halves' causal tails in ONE strided op. Diagonal steps
    either ACT-square then mask their two 128-col windows in place
    (DVE 2X / Pool), or take a fused DVE masked-square exit: one
    cast-multiply by a ones+triangle mask built on device from
    trimask, then one bf16 2X square -- whichever the greedy
    busy-model prefers (the final query block always takes the
    single-op ACT path so the PE tail drains without waiting on a
    2-op DVE chain).
  - accumulate [V|1]^T C' into PSUM [65, 512] (bf16), causally
    narrowed; copy to SBUF bf16 and store raw [Y^T; Z] blocks.
Host epilogue: out = (Y^T / Z)^T per query block (EPS dropped:
Z >= (q.q)^2 >> 1e5*EPS).
"""

import os
import sys

for _p in ("/root/.axon_site", "/root/.axon_site/_ro/trn_rl_repo",
           "/root/.axon_site/_ro/pypackages", "/opt/trn_rl_repo", "/opt/pypackages"):
    if os.path.isdir(_p) and _p not in sys.path:
        sys.path.append(_p)

import ml_dtypes
import numpy as np

import concourse.bacc as bacc
import concourse.mybir as mybir
import concourse.tile as tile
from concourse.bass_utils import run_bass_kernel_spmd

F32 = mybir.dt.float32
BF16 = mybir.dt.bfloat16
NP_BF16 = np.dtype(ml_dtypes.bfloat16)

N_CORES = 8
T = 2048          # tokens
D = 64            # head dim
PAIRS = 2         # (b, h) pairs per core
NKB = T // 128    # 16 key blocks of 128
QB = 512          # query block width
NQB = T // QB     # 4 query blocks

_CACHE = {}


def build_nc():
    nc = bacc.Bacc("TRN2", target_bir_lowering=False, debug=False)

    ins = {}
    outs = {}
    for p in range(PAIRS):
        ins[f"qT2_{p}"] = nc.dram_tensor(f"qT2_{p}", [128, T], BF16, kind="ExternalInput").ap()
        ins[f"kT2_{p}"] = nc.dram_tensor(f"kT2_{p}", [128, T // 2], BF16, kind="ExternalInput").ap()
        ins[f"v1_{p}"] = nc.dram_tensor(f"v1_{p}", [128, NKB * 65], BF16, kind="ExternalInput").ap()
        outs[p] = nc.dram_tensor(f"o{p}", [NQB * 65, QB], BF16, kind="ExternalOutput").ap()
    trimask = nc.dram_tensor("trimask", [128, 128], BF16, kind="ExternalInput").ap()

    # estimated busy-ns per engine, for greedy exit routing; ACT pays
    # a ~1.5us ACT_TABLE_LOAD before its first square, so it starts
    # pre-loaded and the earliest exits route to DVE
    busy = {"A": 1500.0, "V": 0.0, "P": 0.0}

    def add(deltas):
        for k, v in deltas.items():
            busy[k] += v

    def peak(deltas):
        return max(busy[k] + deltas.get(k, 0.0) for k in busy)

    with tile.TileContext(nc) as tc:
        with (
            tc.tile_pool(name="const", bufs=1) as cpool,
            tc.tile_pool(name="persist", bufs=1) as perpool,
            tc.tile_pool(name="cprime", bufs=8) as cppool,
            tc.tile_pool(name="dstage", bufs=4) as dpool,
            tc.tile_pool(name="small", bufs=4) as smpool,
            tc.tile_pool(name="psd", bufs=3, space="PSUM") as psd,
            tc.tile_pool(name="psyz", bufs=2, space="PSUM") as psyz,
        ):
            trimask_sb = cpool.tile([128, 128], BF16, name="trimask_sb")
            # full-width diagonal masks for the fused masked-square DVE
            # exit on partial steps: ones everywhere except a triangular
            # window where each half crosses the diagonal (built on
            # device from trimask during the load dead-time). mA covers
            # the a==2i step's [0:QB] region, mB the a==2i+1 step's
            # [QB/2:QB] region (窗 positions 0/128 within each region).
            mA = cpool.tile([128, 2, QB], BF16, name="mA")
            mB = cpool.tile([128, 2, QB // 2], BF16, name="mB")
            warm = cpool.tile([128, QB], BF16, name="warm")

            qT2 = []
            kT2 = []
            v1 = []
            for p in range(PAIRS):
                qT2.append(perpool.tile([128, T], BF16, name=f"qT2_{p}"))
                kT2.append(perpool.tile([128, T // 2], BF16, name=f"kT2_{p}"))
                v1.append(perpool.tile([128, NKB * 65], BF16, name=f"v1_{p}"))

            # ---- input loads, ordered by first use; the two halves of a
            # pair's first working set ride different queues so the first
            # D' can issue ~0.7us earlier ----
            for p in range(PAIRS):
                dmae = nc.sync if p == 0 else nc.scalar
                dmae2 = nc.scalar if p == 0 else nc.sync
                dmae.dma_start(qT2[p][:, 3 * QB:4 * QB], ins[f"qT2_{p}"][:, 3 * QB:4 * QB])
                # the first D' needs only key blocks 0/1 of its pair, so
                # the first-wave kT2 chunk is just 32KB
                dmae2.dma_start(kT2[p][:, 0:128], ins[f"kT2_{p}"][:, 0:128])
            for p in range(PAIRS):
                dmae = nc.sync if p == 0 else nc.scalar
                dmae.dma_start(kT2[p][:, 128:2 * QB], ins[f"kT2_{p}"][:, 128:2 * QB])
                dmae.dma_start(v1[p][:, 0:8 * 65], ins[f"v1_{p}"][:, 0:8 * 65])
                dmae.dma_start(v1[p][:, 8 * 65:NKB * 65], ins[f"v1_{p}"][:, 8 * 65:NKB * 65])
                dmae.dma_start(qT2[p][:, 2 * QB:3 * QB], ins[f"qT2_{p}"][:, 2 * QB:3 * QB])
                dmae.dma_start(qT2[p][:, QB:2 * QB], ins[f"qT2_{p}"][:, QB:2 * QB])
                dmae.dma_start(qT2[p][:, 0:QB], ins[f"qT2_{p}"][:, 0:QB])
            # trimask is first needed by the earliest diagonal mask (~15us
            # in); loading it last keeps the critical first-wave loads front
            nc.sync.dma_start(trimask_sb[:], trimask[:])
            for m_ in (mA, mB):
                nc.vector.memset(m_[:], 1.0)
                nc.vector.tensor_copy(m_[:, 0, 0:128], trimask_sb[:])
                nc.vector.tensor_copy(m_[:, 1, 128:256], trimask_sb[:])

            # ---- PE pre-warm: the PE p-state ramps to full clock only
            # after ~3us of continuous execution, so run ~12 throwaway
            # row-tiled matmuls on a zeroed scratch tile into the psd
            # bufs during the initial load dead-time, sized to end right
            # as the first real D' inputs arrive (results are
            # overwritten by the first real start=True writes; as a
            # side effect every psd buf's odd half is written, so the
            # fused partial-step exits never read undefined PSUM) ----
            nc.vector.memset(warm[:], 0.0)
            _pre = [psd.tile([128, 2, QB], F32, name="psAB", tag="psd")
                    for _ in range(3)]
            for _rep in range(2):
                for t_ in _pre:
                    nc.tensor.matmul(t_[:, 0, 0:QB], warm[0:64, 0:128],
                                     warm[0:64, 0:QB], start=True, stop=True,
                                     tile_position=(0, 0), skip_group_check=True)
                    nc.tensor.matmul(t_[:, 1, 0:QB], warm[64:128, 0:128],
                                     warm[64:128, 0:QB], start=True, stop=True,
                                     tile_position=(64, 0), skip_group_check=True)

            def psd_tile():
                if _pre:
                    return _pre.pop(0)
                return psd.tile([128, 2, QB], F32, name="psAB", tag="psd")

            tails = {}

            def exit_square(dst, src, cols):
                """PSUM->SBUF squaring exit, greedily balanced with
                measured per-op costs (ns). ACT squares directly; DVE
                copies (cast) then bf16-muls at 2X (a 2-PSUM-operand
                tensor_tensor is rejected by the BIR verifier)."""
                optA = {"A": cols * 0.833 + 396}
                optV = {"V": cols * 1.562 + 340}
                best = min((optA, optV), key=peak)
                add(best)
                if best is optA:
                    nc.scalar.square(dst, src)
                else:
                    dstg = dpool.tile([128, 2 * QB], BF16, name="dstg", tag="dstg")
                    stg = dstg[:, 0:cols]
                    nc.vector.tensor_copy(stg, src)
                    nc.vector.tensor_mul(dst, stg, stg)

            def exit_copy(dst, src, cols):
                optA = {"A": cols * 0.833 + 396}
                optV = {"V": cols * 1.042 + 170}
                best = min((optA, optV), key=peak)
                add(best)
                if best is optA:
                    nc.scalar.copy(dst, src)
                else:
                    nc.vector.tensor_copy(dst, src)

            def mask_window(c2, par, w):
                """in-place triangular mask on a 128-col diagonal window;
                DVE 2X bf16 unless Pool is the lighter engine."""
                optV = {"V": 128 * 0.52 + 170}
                optP = {"P": 128 * 3.12 + 250}
                best = min((optV, optP), key=peak)
                add(best)
                eng = nc.vector if best is optV else nc.gpsimd
                eng.tensor_mul(c2[:, par, w:w + 128], c2[:, par, w:w + 128],
                               trimask_sb[:])

            def emit_dpair(p, i, a, tail=False):
                """D'[k, q] for key blocks (2a, 2a+1) vs query block i,
                narrowed to causal cols; returns (c2 tile, w0E, w0O)."""
                kcols = slice(a * 128, (a + 1) * 128)
                psAB = psd_tile()
                w0E = max(0, 128 * (2 * a) - QB * i)
                w0O = max(0, 128 * (2 * a + 1) - QB * i)
                nc.tensor.matmul(
                    psAB[:, 0, w0E:QB], kT2[p][0:64, kcols],
                    qT2[p][0:64, i * QB + w0E:(i + 1) * QB],
                    start=True, stop=True, tile_position=(0, 0),
                    skip_group_check=True,
                )
                nc.tensor.matmul(
                    psAB[:, 1, w0O:QB], kT2[p][64:128, kcols],
                    qT2[p][64:128, i * QB + w0O:(i + 1) * QB],
                    start=True, stop=True, tile_position=(64, 0),
                    skip_group_check=True,
                )
                c2 = cppool.tile([128, 2, QB], BF16, name="c2", tag="cp")
                # one strided op covers both halves' causal tails; for
                # partial (diagonal) steps the odd half's 128 cols
                # [w0E:w0O] hold stale values of an earlier full step
                # (i descends, so each psd buf is matmul-written
                # full-width twice before any partial step) and are
                # never consumed by the narrowed CV
                cols = 2 * (QB - w0E)
                diag = w0O == w0E + 128 and 0 <= 2 * a - 4 * i < 4
                if diag and tail:
                    # the kernel's last CVs sit right behind these
                    # exits; take the single-op ACT path (lowest
                    # latency) and mask on DVE so the PE tail drains
                    # without waiting on a 2-op DVE chain
                    add({"A": cols * 0.833 + 396, "V": 2 * 237.0})
                    nc.scalar.square(c2[:, :, w0E:QB], psAB[:, :, w0E:QB])
                elif diag:
                    # partial step: either ACT square + 2 strided DVE
                    # mask windows, or ONE DVE masked-square (cast-mul
                    # by the ones+triangle mask, then bf16 2X square)
                    optA = {"A": cols * 0.833 + 396, "V": 2 * 237.0}
                    optVm = {"V": cols * 1.563 + 340}
                    best = min((optA, optVm), key=peak)
                    add(best)
                    if best is optVm:
                        m_ = mA if w0E == 0 else mB
                        dstg = dpool.tile([128, 2 * QB], BF16, name="dstg",
                                          tag="dstg")
                        stg = dstg[:, 0:cols]
                        nc.vector.tensor_mul(stg, psAB[:, :, w0E:QB],
                                             m_[:, :, :])
                        nc.vector.tensor_mul(c2[:, :, w0E:QB], stg, stg)
                        return c2, w0E, w0O
                    nc.scalar.square(c2[:, :, w0E:QB], psAB[:, :, w0E:QB])
                else:
                    exit_square(c2[:, :, w0E:QB], psAB[:, :, w0E:QB], cols)
                for par, w0 in ((0, w0E), (1, w0O)):
                    r = 2 * a + par - (QB // 128) * i
                    if 0 <= r < QB // 128:
                        mask_window(c2, par, w0)
                return c2, w0E, w0O

            def emit_cv(p, i, a, cur):
                c2, w0E, w0O = cur
                yzp = tails[(p, i)]
                vE = v1[p][:, (2 * a) * 65:(2 * a + 1) * 65]
                vO = v1[p][:, (2 * a + 1) * 65:(2 * a + 2) * 65]
                nc.tensor.matmul(
                    yzp[:, w0E:QB], vE, c2[:, 0, w0E:QB],
                    start=(a == 0), stop=False, skip_group_check=True,
                )
                nc.tensor.matmul(
                    yzp[:, w0O:QB], vO, c2[:, 1, w0O:QB],
                    start=False, stop=(a == 2 * i + 1), skip_group_check=True,
                )

            def emit_tail(p, i):
                yzs = smpool.tile([65, QB], BF16, name="yzs", tag="yzs")
                exit_copy(yzs[:], tails[(p, i)][:], QB)
                dmae = nc.sync if p == 0 else nc.scalar
                dmae.dma_start(outs[p][i * 65:(i + 1) * 65, :], yzs[:])

            AHEAD = 2  # super-steps between a dpair's D' and its CV
            seq = [(i, a) for i in (3, 2, 1, 0) for a in range(2 * i + 2)]
            dp = {}

            def emit_d(p, j):
                dp[(p, j)] = emit_dpair(p, *seq[j], tail=j >= len(seq) - 2)

            def emit_c(p, j):
                i, a = seq[j]
                cur = dp.pop((p, j))
                if a == 0:
                    tails[(p, i)] = psyz.tile(
                        [65, QB], F32, name=f"yzp_{p}_{i}", tag="yzp"
                    )
                emit_cv(p, i, a, cur)
                if a == 2 * i + 1:
                    emit_tail(p, i)

            # D' units run AHEAD super-steps before their CV so the PE
            # always has an independent matmul between dependent ones and
            # the exit engines get a full pipeline of slack
            for j in range(len(seq) + AHEAD):
                for p in range(PAIRS):
                    if j < len(seq):
                        emit_d(p, j)
                for p in range(PAIRS):
                    if j >= AHEAD:
                        emit_c(p, j - AHEAD)

    nc.compile()
    return nc


def _shard_inputs(Q, K, V):
    """Per-core in_maps; core c gets global (b,h) pairs 2c and 2c+1.

    Host-side layout prep: bf16 cast + transpose into the exact SBUF
    layouts the device consumes (no device-side transposes/casts).
    """
    Q = np.asarray(Q, dtype=np.float32)
    K = np.asarray(K, dtype=np.float32)
    V = np.asarray(V, dtype=np.float32)
    b, t, h, d = Q.shape
    trimask = np.triu(np.ones((128, 128), dtype=np.float32)).astype(NP_BF16)
    in_maps = []
    for c in range(N_CORES):
        m = {"trimask": trimask}
        for p in range(PAIRS):
            g = PAIRS * c + p
            bb, hh = divmod(g, h)
            qT = np.ascontiguousarray(Q[bb, :, hh, :].T).astype(NP_BF16)  # [64, 2048]
            kT = np.ascontiguousarray(K[bb, :, hh, :].T).astype(NP_BF16)
            vp = V[bb, :, hh, :].astype(NP_BF16)                          # [2048, 64]
            m[f"qT2_{p}"] = np.concatenate([qT, qT], axis=0)              # [128, 2048]
            kT2 = np.empty((128, T // 2), dtype=NP_BF16)
            kTb = kT.reshape(64, NKB, 128)
            kT2[0:64] = kTb[:, 0::2, :].reshape(64, T // 2)
            kT2[64:128] = kTb[:, 1::2, :].reshape(64, T // 2)
            m[f"kT2_{p}"] = kT2
            v1 = np.ones((128, NKB, 65), dtype=NP_BF16)
            v1[:, :, 0:64] = vp.reshape(NKB, 128, 64).transpose(1, 0, 2)
            m[f"v1_{p}"] = v1.reshape(128, NKB * 65)
        in_maps.append(m)
    return in_maps


def kernel(Q, K, V, chunk_count, trace=False):
    Q = np.asarray(Q)
    b, t, h, d = Q.shape
    assert (b, t, h, d) == (2, T, 8, D), (b, t, h, d)
    assert T % int(chunk_count) == 0

    if "nc" not in _CACHE:
        _CACHE["nc"] = build_nc()
    nc = _CACHE["nc"]

    in_maps = _shard_inputs(Q, K, V)
    res = run_bass_kernel_spmd(nc, in_maps, core_ids=list(range(N_CORES)), trace=trace)

    out = np.empty((b, t, h, d), dtype=np.float32)
    for c in range(N_CORES):
        for p in range(PAIRS):
            g = PAIRS * c + p
            bb, hh = divmod(g, h)
            arr = np.asarray(res.results[c][f"o{p}"]).astype(np.float32)
            arr = arr.reshape(NQB, 65, QB)
            y = arr[:, :64, :]                      # [NQB, 64, QB]
            z = arr[:, 64, :]                       # [NQB, QB]
            outp = (y / z[:, None, :]).transpose(0, 2, 1).reshape(T, D)
            out[bb, :, hh, :] = outp
    if trace:
        return out, res
    return out
